# revision 22
# baseline (speedup 1.0000x reference)
"""DimeNet edge_init (DimePredictor) Bass/Trainium2 kernel.

Strategy (8 NeuronCores), v6: dma_gather (custom ucode) spread over all
4 SWDGE queues.

v6 finding: with one SWDGE queue the gather DMA is latency-bound (the
per-queue descriptor ring drains serially; ~13ms measured on a shared
trn2.8x1).  Issuing the 8 gathers of each supertile round-robin across
the 4 SWDGE queues gives the DMA engines 4 independent rings to
interleave, cutting HW time ~4.5x (13ms -> 2.9ms).  single_packet=True
hangs this runtime (desc/ring accounting mismatch) - keep False.
Per-bucket tight num_idxs (static idx trim) was tried and measured
slower; constant NB keeps full-width gathers and no tail memsets.

Hardware facts that shaped this design:
  - indirect_dma_start honors only ONE offset per partition per instruction
    (~1.2us SWDGE overhead per 128 gathered rows -> ~11ms for 8M rows): the
    baseline's bottleneck.  Batching offsets is a simulator-only fiction.
  - dma_gather (InstDMAGatherAnt, the MoE ucode path) gathers up to 8191
    rows per instruction, but: int16 indices (windows of 32768 table rows),
    256-byte element/stride granularity, idx array replicated over the 8 Q7
    cores as [128, n/16].

Design:
  - Host (layout only): pad table to 2^20 rows; fused row tbl64[e] =
    [rbf_env(42) | o(3) | pad] as 64 f32 (256B).  Bucket all 4M triplets by
    (src>>15, dst>>15) into 32x32 buckets; core c owns src-windows
    4c..4c+3 (128 buckets/core), each padded to NB=4224 slots (idx 0 pads).
  - Device, per bucket: dma_gather ft row by local src (window slice of
    tbl64), dma_gather same-table row by local dst (for o[dst]).  4 buckets
    form one compute supertile [128, 132 lanes]:
      c = (R1.R2) * rsqrt(|R1|^2|R2|^2); scaled Legendre G_l;
      sbf = rbf[src] * repeat(qscale_l*G_l, 6)  (fused [128,132,7,6] mult)
    Output written bf16 (tolerance 2e-2; bf16 adds <0.8% error).
  - Host: invert the bucket permutation, upcast to f32.
"""
import math
import numpy as np

NUM_SPHERICAL = 7
NUM_RADIAL = 6
D_OUT = NUM_SPHERICAL * NUM_RADIAL  # 42
E_ROWS = 1_000_000
T_FULL = 4_000_000
N_CORES = 8

W_BITS = 15
WIN = 1 << W_BITS                 # 32768 rows per index window
N_WIN = 32                        # windows over the padded table
E_PAD = N_WIN * WIN               # 1,048,576
TBL_W = 64                        # f32 words per row (256B)
O_OFF = 42                        # o starts after the 42 rbf words

SW_PER_CORE = N_WIN // N_CORES    # 4 src windows per core
NB = 4608                         # idx slots per bucket (36 * 128)
NBLK = NB // 128                  # 36 blocks per bucket
N_BUCKET = SW_PER_CORE * N_WIN    # 128 buckets per core
SB = 4                            # buckets per compute supertile
N_SUPER = N_BUCKET // SB          # 32 supertiles
LANES = SB * NBLK                 # 132 columns per supertile
T_SLOT = N_BUCKET * NB            # 540,672 slots per core

_CACHE = {}


def build_program(n_cores=N_CORES, repeat=1, n_win=N_WIN, win_bits=W_BITS,
                  nb=NB, sw_per_core=SW_PER_CORE, sb=SB, salt=0,
                  zero_tiles=False, n_queues=4, single_packet=False, deep=2,
                  no_gather=False, nidx_list=None):
    import concourse.bacc as bacc
    import concourse.bass as bass
    import concourse.tile as tile
    import concourse.mybir as mybir

    f32 = mybir.dt.float32
    bf16 = mybir.dt.bfloat16
    i16 = mybir.dt.int16
    win = 1 << win_bits
    e_pad = n_win * win
    nblk = nb // 128
    n_bucket = sw_per_core * n_win
    n_super = n_bucket // sb
    lanes = sb * nblk
    t_slot = n_bucket * nb
    if nidx_list is None:
        nidx_list = [nb] * n_bucket
    assert len(nidx_list) == n_bucket and all(
        16 <= x <= nb and x % 16 == 0 for x in nidx_list)
    off = np.concatenate([[0], np.cumsum([x // 16 for x in nidx_list])])
    idx_cols = int(off[-1])

    nc = bacc.Bacc("TRN2", target_bir_lowering=False, debug=False, num_devices=n_cores,
                   num_swdge_queues=n_queues)
    # full table (dst gathers index global windows; same program all cores)
    tbl = nc.dram_tensor("tblq", [e_pad, TBL_W], f32, kind="ExternalInput").ap()
    # this core's src windows (data differs per core, program identical)
    tbls = nc.dram_tensor("tbls", [sw_per_core * win, TBL_W], f32,
                          kind="ExternalInput").ap()
    # idx arrays: packed [128, idx_cols] int16; bucket b at cols off[b]:off[b+1]
    # (16-partition wrap, replicated 8x along partitions for the Q7 cores)
    sidx = nc.dram_tensor("sidx", [128, idx_cols], i16,
                          kind="ExternalInput").ap()
    didx = nc.dram_tensor("didx", [128, idx_cols], i16,
                          kind="ExternalInput").ap()
    bcount = nc.dram_tensor("bcount", [1, n_bucket], mybir.dt.int32,
                            kind="ExternalInput").ap()
    # salt: unique shape per (repeat, salt) so the XLA module hash (which
    # ignores the embedded BIR!) can never collide across program variants
    saltt = nc.dram_tensor("salt", [1, 8 + repeat + salt], f32,
                           kind="ExternalInput").ap()
    out = nc.dram_tensor("out", [128, t_slot * D_OUT // 128], bf16,
                         kind="ExternalOutput").ap()

    # scaled Legendre recurrence: G_l = c*G_{l-1} - b2_l*G_{l-2}
    g = [1.0, 1.0]
    for l in range(2, NUM_SPHERICAL):
        g.append((2 * l - 1) / l * g[-1])
    b2 = {l: ((l - 1) / l) * g[l - 2] / ((2 * l - 1) / l * g[l - 1])
          for l in range(2, NUM_SPHERICAL)}
    coef = [float(np.sqrt((2 * l + 1) / (4.0 * np.pi)).astype(np.float32))
            for l in range(NUM_SPHERICAL)]
    qscale = [coef[l] * g[l] for l in range(NUM_SPHERICAL)]

    mul = mybir.AluOpType.mult
    add = mybir.AluOpType.add
    sub = mybir.AluOpType.subtract

    with tile.TileContext(nc) as tc:
        with tc.tile_pool(name="saltp", bufs=1) as saltp, \
             tc.tile_pool(name="idxp", bufs=2 * deep) as idxp, \
             tc.tile_pool(name="ftp", bufs=deep) as ftp, \
             tc.tile_pool(name="odp", bufs=deep) as odp, \
             tc.tile_pool(name="otp", bufs=2) as otp, \
             tc.tile_pool(name="tmp", bufs=2) as tmp:
            st = saltp.tile([1, 8 + repeat + salt], f32)
            nc.sync.dma_start(st[:], saltt[:])
            cnt_t = saltp.tile([1, n_bucket], mybir.dt.int32)
            nc.sync.dma_start(cnt_t[:], bcount[:])
            nreg = nc.gpsimd.alloc_register("nidx")

            for _rep in range(repeat):
                for sup in range(n_super):
                    ft = ftp.tile([128, lanes * TBL_W], f32)
                    od = odp.tile([128, lanes * TBL_W], f32)
                    if zero_tiles:   # sim only: pad lanes stay finite
                        nc.vector.memset(ft[:], 0.0)
                        nc.vector.memset(od[:], 0.0)
                    ft3 = ft[:].rearrange("p (k f) -> p k f", f=TBL_W)
                    od3 = od[:].rearrange("p (k f) -> p k f", f=TBL_W)
                    for i in range(sb):
                        b = sup * sb + i
                        swi = b // n_win          # src window (core-local)
                        dwi = b % n_win           # dst window
                        nib = int(nidx_list[b])
                        wb = nib // 16
                        ob = int(off[b])
                        w128 = (nib + 127) // 128
                        if w128 < nblk:   # lanes the gather won't write
                            nc.vector.memset(
                                ft3[:, i * nblk + w128:(i + 1) * nblk, :], 0.0)
                            nc.vector.memset(
                                od3[:, i * nblk + w128:(i + 1) * nblk, :], 0.0)
                        sit = idxp.tile([128, nb // 16], i16, tag=f"s{i}")
                        dit = idxp.tile([128, nb // 16], i16, tag=f"d{i}")
                        nc.sync.dma_start(sit[:, :wb], sidx[:, ob:ob + wb])
                        nc.sync.dma_start(dit[:, :wb], didx[:, ob:ob + wb])
                        nc.gpsimd.reg_load(nreg, cnt_t[0:1, b:b + 1])
                        if no_gather:
                            continue
                        nc.gpsimd.dma_gather(
                            out_ap=ft3[:, i * nblk:i * nblk + (nib + 127) // 128, :],
                            in_ap=tbls[swi * win:(swi + 1) * win, :],
                            idxs_ap=sit[:, :wb],
                            num_idxs=nib,
                            num_idxs_reg=nreg,
                            elem_size=TBL_W,
                            single_packet=single_packet,
                            queue_num=(2 * i) % n_queues,
                        )
                        nc.gpsimd.dma_gather(
                            out_ap=od3[:, i * nblk:i * nblk + (nib + 127) // 128, :],
                            in_ap=tbl[dwi * win:(dwi + 1) * win, :],
                            idxs_ap=dit[:, :wb],
                            num_idxs=nib,
                            num_idxs_reg=nreg,
                            elem_size=TBL_W,
                            single_packet=single_packet,
                            queue_num=(2 * i + 1) % n_queues,
                        )

                    K = lanes
                    R1 = ft3[:, :, O_OFF:O_OFF + 3]
                    R2 = od3[:, :, O_OFF:O_OFF + 3]
                    rbf4 = ft3[:, :, 0:D_OUT].rearrange(
                        "p k (l r) -> p k l r", r=NUM_RADIAL)

                    m = tmp.tile([128, K * 3], f32, tag="m")
                    m3 = m[:].rearrange("p (k f) -> p k f", f=3)
                    sc = tmp.tile([128, K * 8], f32, tag="sc")
                    dot = sc[:, 0 * K:1 * K]
                    n1 = sc[:, 1 * K:2 * K]
                    n2 = sc[:, 2 * K:3 * K]
                    cc = sc[:, 3 * K:4 * K]      # becomes G1
                    t4 = sc[:, 4 * K:5 * K]
                    t5 = sc[:, 5 * K:6 * K]
                    t6 = sc[:, 6 * K:7 * K]
                    t7 = sc[:, 7 * K:8 * K]
                    w = tmp.tile([128, K * NUM_SPHERICAL], f32, tag="w")

                    nc.vector.tensor_tensor(out=m3[:], in0=R1, in1=R2, op=mul)
                    nc.vector.tensor_tensor(out=dot, in0=m[:, 0::3], in1=m[:, 1::3], op=add)
                    nc.vector.tensor_tensor(out=dot, in0=dot, in1=m[:, 2::3], op=add)
                    nc.vector.tensor_tensor(out=m3[:], in0=R1, in1=R1, op=mul)
                    nc.vector.tensor_tensor(out=n1, in0=m[:, 0::3], in1=m[:, 1::3], op=add)
                    nc.vector.tensor_tensor(out=n1, in0=n1, in1=m[:, 2::3], op=add)
                    nc.vector.tensor_tensor(out=m3[:], in0=R2, in1=R2, op=mul)
                    nc.vector.tensor_tensor(out=n2, in0=m[:, 0::3], in1=m[:, 1::3], op=add)
                    nc.vector.tensor_tensor(out=n2, in0=n2, in1=m[:, 2::3], op=add)
                    # c = dot * rsqrt(n1*n2): ACT sqrt seed + one Newton step
                    nc.vector.tensor_tensor(out=t4, in0=n1, in1=n2, op=mul)
                    nc.scalar.sqrt(out=t5, in_=t4)
                    nc.vector.reciprocal(out=t5, in_=t5)
                    nc.vector.tensor_tensor(out=t6, in0=t5, in1=t5, op=mul)
                    nc.vector.tensor_tensor(out=t6, in0=t6, in1=t4, op=mul)
                    nc.vector.tensor_scalar(out=t6, in0=t6, scalar1=-0.5, scalar2=1.5,
                                            op0=mul, op1=add)
                    nc.vector.tensor_tensor(out=t5, in0=t5, in1=t6, op=mul)
                    nc.vector.tensor_tensor(out=cc, in0=dot, in1=t5, op=mul)

                    def wl(l):
                        return w[:, l * K:(l + 1) * K]

                    nc.vector.memset(wl(0), float(qscale[0]))
                    nc.vector.tensor_scalar(out=wl(1), in0=cc,
                                            scalar1=float(qscale[1]), scalar2=None,
                                            op0=mul)
                    G2, G3, G4, G5, G6 = t6, t7, t4, t5, t6
                    nc.vector.tensor_tensor(out=G2, in0=cc, in1=cc, op=mul)
                    nc.vector.tensor_scalar(out=G2, in0=G2, scalar1=float(-b2[2]),
                                            scalar2=None, op0=add)
                    nc.vector.tensor_scalar(out=wl(2), in0=G2,
                                            scalar1=float(qscale[2]), scalar2=None,
                                            op0=mul)
                    nc.vector.tensor_scalar(out=G3, in0=G2, scalar1=float(-b2[3]),
                                            scalar2=None, op0=add)
                    nc.vector.tensor_tensor(out=G3, in0=G3, in1=cc, op=mul)
                    nc.vector.tensor_scalar(out=wl(3), in0=G3,
                                            scalar1=float(qscale[3]), scalar2=None,
                                            op0=mul)
                    nc.vector.tensor_tensor(out=G4, in0=cc, in1=G3, op=mul)
                    nc.vector.tensor_scalar(out=G2, in0=G2, scalar1=float(b2[4]),
                                            scalar2=None, op0=mul)
                    nc.vector.tensor_tensor(out=G4, in0=G4, in1=G2, op=sub)
                    nc.vector.tensor_scalar(out=wl(4), in0=G4,
                                            scalar1=float(qscale[4]), scalar2=None,
                                            op0=mul)
                    nc.vector.tensor_tensor(out=G5, in0=cc, in1=G4, op=mul)
                    nc.vector.tensor_scalar(out=G3, in0=G3, scalar1=float(b2[5]),
                                            scalar2=None, op0=mul)
                    nc.vector.tensor_tensor(out=G5, in0=G5, in1=G3, op=sub)
                    nc.vector.tensor_scalar(out=wl(5), in0=G5,
                                            scalar1=float(qscale[5]), scalar2=None,
                                            op0=mul)
                    nc.vector.tensor_tensor(out=G6, in0=cc, in1=G5, op=mul)
                    nc.vector.tensor_scalar(out=G4, in0=G4, scalar1=float(b2[6]),
                                            scalar2=None, op0=mul)
                    nc.vector.tensor_tensor(out=G6, in0=G6, in1=G4, op=sub)
                    nc.vector.tensor_scalar(out=wl(6), in0=G6,
                                            scalar1=float(qscale[6]), scalar2=None,
                                            op0=mul)

                    ot = otp.tile([128, K * D_OUT], bf16)
                    ot4 = ot[:].rearrange("p (k l r) -> p k l r",
                                          l=NUM_SPHERICAL, r=NUM_RADIAL)
                    wb = (w[:].rearrange("p (l k) -> p k l", k=K)
                          .rearrange("p k (l o) -> p k l o", o=1)
                          .to_broadcast([128, K, NUM_SPHERICAL, NUM_RADIAL]))
                    nc.vector.tensor_tensor(out=ot4, in0=rbf4, in1=wb, op=mul)

                    nc.sync.dma_start(
                        out[:, sup * K * D_OUT:(sup + 1) * K * D_OUT], ot[:])

    nc.compile()
    return nc


def _get_runner(nc, n_cores):
    """Build a jitted SPMD executor for the compiled Bass program."""
    import jax
    import jax.numpy as jnp
    from jax.sharding import Mesh, PartitionSpec, NamedSharding
    from jax.experimental.shard_map import shard_map
    import concourse.mybir as mybir
    from concourse.bass2jax import _bass_exec_p, install_neuronx_cc_hook, partition_id_tensor

    install_neuronx_cc_hook()
    partition_name = nc.partition_id_tensor.name if nc.partition_id_tensor else None
    in_names, out_names, out_avals = [], [], []
    for alloc in nc.m.functions[0].allocations:
        if not isinstance(alloc, mybir.MemoryLocationSet):
            continue
        name = alloc.memorylocations[0].name
        if alloc.kind == "ExternalInput":
            if name != partition_name:
                in_names.append(name)
        elif alloc.kind == "ExternalOutput":
            out_names.append(name)
            out_avals.append(jax.core.ShapedArray(
                tuple(alloc.tensor_shape), mybir.dt.np(alloc.dtype)))
    n_params = len(in_names)
    n_outs = len(out_avals)
    all_in_names = in_names + out_names
    if partition_name is not None:
        all_in_names = all_in_names + [partition_name]
    donate = tuple(range(n_params, n_params + n_outs))

    def _body(*args):
        operands = list(args)
        if partition_name is not None:
            operands.append(partition_id_tensor())
        outs = _bass_exec_p.bind(
            *operands,
            out_avals=tuple(out_avals),
            in_names=tuple(all_in_names),
            out_names=tuple(out_names),
            lowering_input_output_aliases=(),
            sim_require_finite=False,
            sim_require_nnan=False,
            nc=nc,
        )
        return tuple(outs)

    try:
        devices = jax.devices("axon")[:n_cores]
    except RuntimeError:
        devices = jax.devices()[:n_cores]
    mesh = Mesh(np.asarray(devices), ("core",))
    sharded = jax.jit(
        shard_map(_body, mesh=mesh,
                  in_specs=(PartitionSpec("core"),) * (n_params + n_outs),
                  out_specs=(PartitionSpec("core"),) * n_outs,
                  check_rep=False),
        donate_argnums=donate,
        keep_unused=True,
    )
    shard0 = NamedSharding(mesh, PartitionSpec("core"))

    def make_zeros():
        return [
            jax.jit(
                lambda shape=av.shape, dt=av.dtype: jnp.zeros(
                    (n_cores * shape[0],) + tuple(shape[1:]), dt),
                out_shardings=shard0,
            )()
            for av in out_avals
        ]

    return sharded, in_names, out_names, out_avals, shard0, make_zeros


def pack_table(o, rbf_env, e_pad=E_PAD):
    tblq = np.zeros((e_pad, TBL_W), dtype=np.float32)
    n = o.shape[0]
    tblq[:n, :D_OUT] = rbf_env
    tblq[:n, O_OFF:O_OFF + 3] = o
    return tblq


def wrap_idx_packed(local_idx, nidx_list, off):
    """[n_bucket, NB] int16 -> packed [128, idx_cols] (16-part wrap, 8x rep)."""
    idx_cols = int(off[-1])
    arr = np.full((16, idx_cols), -1, dtype=np.int16)
    for b, nib in enumerate(nidx_list):
        wb = nib // 16
        blk = local_idx[b, :nib].reshape(wb, 16).T     # [16, wb]
        arr[:, off[b]:off[b] + wb] = blk
    a = np.broadcast_to(arr[None, :, :], (8, 16, idx_cols))
    return np.ascontiguousarray(a.reshape(128, idx_cols))


def prep_inputs(o, rbf_env, src_idx, dst_idx, salt_width=9):
    """Host-side layout-only prep: bucket/pad/pack. Returns concat arrays,
    the slot->triplet permutation, and the per-bucket static num_idxs."""
    o = np.asarray(o, dtype=np.float32)
    rbf = np.asarray(rbf_env, dtype=np.float32)
    src = np.asarray(src_idx).astype(np.int64)
    dst = np.asarray(dst_idx).astype(np.int64)
    assert o.shape == (E_ROWS, 3) and rbf.shape == (E_ROWS, D_OUT)

    tblq = pack_table(o, rbf)

    bucket = (src >> W_BITS) * N_WIN + (dst >> W_BITS)    # [T] in [0, 1024)
    order = np.argsort(bucket, kind="stable")
    counts = np.bincount(bucket, minlength=N_WIN * N_WIN)
    assert counts.max() <= NB, f"bucket overflow: {counts.max()} > {NB}"
    starts = np.zeros(N_WIN * N_WIN, dtype=np.int64)
    starts[1:] = np.cumsum(counts)[:-1]

    # static num_idxs per bucket position: max count across cores, /16 aligned
    cnt_cb = np.zeros((N_CORES, N_BUCKET), dtype=np.int64)
    for c in range(N_CORES):
        for b in range(N_BUCKET):
            gb = (c * SW_PER_CORE + b // N_WIN) * N_WIN + (b % N_WIN)
            cnt_cb[c, b] = counts[gb]
    # constant static num_idxs: tight per-bucket values were measured slower
    # (variable-width idx DMAs + tail memsets cost more than the Q7 unpack
    # savings); cnt_cb kept for the overflow assert below
    assert cnt_cb.max() <= NB
    nidx_list = (NB,) * N_BUCKET
    off = np.concatenate([[0], np.cumsum([x // 16 for x in nidx_list])])

    sidx_all = np.full((N_CORES, N_BUCKET, NB), -1, dtype=np.int16)
    didx_all = np.full((N_CORES, N_BUCKET, NB), -1, dtype=np.int16)
    bcnt = np.ones((N_CORES, 1, N_BUCKET), dtype=np.int32)
    perm = np.full((N_CORES, N_BUCKET, NB), -1, dtype=np.int64)
    mask = (1 << W_BITS) - 1
    for c in range(N_CORES):
        for b in range(N_BUCKET):
            gb = (c * SW_PER_CORE + b // N_WIN) * N_WIN + (b % N_WIN)
            n = counts[gb]
            tr = order[starts[gb]:starts[gb] + n]
            sidx_all[c, b, :n] = (src[tr] & mask).astype(np.int16)
            didx_all[c, b, :n] = (dst[tr] & mask).astype(np.int16)
            perm[c, b, :n] = tr
            if n == 0:   # ucode needs >=1 valid idx per call
                sidx_all[c, b, 0] = 0
                didx_all[c, b, 0] = 0
                n = 1
            bcnt[c, 0, b] = n

    concat = {
        "tblq": np.concatenate([tblq] * N_CORES, axis=0),
        "tbls": np.concatenate(
            [tblq[c * SW_PER_CORE * WIN:(c + 1) * SW_PER_CORE * WIN]
             for c in range(N_CORES)], axis=0),
        "sidx": np.concatenate(
            [wrap_idx_packed(sidx_all[c], nidx_list, off)
             for c in range(N_CORES)], axis=0),
        "didx": np.concatenate(
            [wrap_idx_packed(didx_all[c], nidx_list, off)
             for c in range(N_CORES)], axis=0),
        "bcount": np.concatenate([bcnt[c] for c in range(N_CORES)], axis=0),
        "salt": np.zeros((N_CORES, salt_width), dtype=np.float32),
    }
    return concat, perm.reshape(-1), nidx_list


def assemble_output(out_concat, perm_flat):
    """out_concat: [N_CORES*128, T_SLOT*42/128] bf16 -> [T_FULL, 42] f32."""
    res = np.empty((T_FULL, D_OUT), dtype=np.float32)
    for c in range(N_CORES):
        blk = np.asarray(out_concat[c * 128:(c + 1) * 128])
        # [128, n_super, lanes, 42] ; slot i of supertile = lane*128 + p
        a = blk.reshape(128, N_SUPER, LANES, D_OUT).transpose(1, 2, 0, 3)
        a = a.reshape(T_SLOT, D_OUT)  # slot-ordered (bucket-major)
        p = perm_flat[c * T_SLOT:(c + 1) * T_SLOT]
        valid = p >= 0
        res[p[valid]] = a[valid].astype(np.float32)
    return res


def kernel(o, rbf_env, src_idx, dst_idx):
    import jax

    concat, perm_flat, nidx_list = prep_inputs(o, rbf_env, src_idx, dst_idx)
    if _CACHE.get("nidx") != nidx_list:
        _CACHE["prog"] = build_program(nidx_list=nidx_list)
        _CACHE["runner"] = _get_runner(_CACHE["prog"], N_CORES)
        _CACHE["nidx"] = nidx_list
    sharded, in_names, out_names, out_avals, shard0, make_zeros = _CACHE["runner"]

    dev_in = [jax.device_put(concat[name], shard0) for name in in_names]
    outs = sharded(*dev_in, *make_zeros())
    jax.block_until_ready(outs)
    out_concat = np.asarray(outs[out_names.index("out")])
    return assemble_output(out_concat, perm_flat)

